# revision 10
# baseline (speedup 1.0000x reference)
# Trainium2 Bass kernel for FC_STGNN pedestrian edge-MLP (gnn_message_passing).
#
# Reference computation (BS=4, N=512, EMB=64):
#   h    = leaky(nf @ fc1_w + fc1_b)            [4,512,128]
#   emb  = leaky(h @ fc2_w + fc2_b)             [4,512,64]
#   edge_in[b,i,j] = [emb_i - emb_j, emb_i*emb_j]
#   eh   = relu(edge_in @ e1_w + e1_b)          [4,512,512,64]
#   logits = eh @ e2_w + e2_b                   [4,512,512,2]
#   edge_prob = softmax(logits)                 -> adjacency = p1, prediction = off-diag (p0,p1)
#
# Kernel restructuring:
#   * softmax over 2 classes == sigmoid of the logit difference:
#       d = eh @ (e2_w[:,1]-e2_w[:,0]) + (e2_b[1]-e2_b[0]);  p1 = sigmoid(d), p0 = sigmoid(-d)
#   * pre-activation factorization (W1 = e1_w[:64], W2 = e1_w[64:]):
#       pre[i,j,:] = emb_j @ (diag(emb_i) @ W2 - W1) + (emb_i @ W1 + e1_b)
#     For each pair of output rows (i, i+1) we build a [64,128] weight (two fused
#     vector ops), run one [64x128x512] fp32r matmul against the shared embT, and
#     absorb the per-i constant as the relu's per-partition bias (C2 trick below).
#   * the logit reduction runs with eh as the (bf16, FWL) stationary operand so the
#     [2,512] per-pair logits land as columns of a shared PSUM bank; sigmoid is then
#     batched over 64 pairs and PE-transposed back to row-major output chunks.
#
# Sharding: 8 cores; core c -> batch b=c//2, i-rows [i0, i0+256) with i0=256*(c%2).
# The per-core program is identical: the j axis is ROTATED by -i0 on the host
# (nfT input is np.roll'ed), so device columns j' correspond to real j=(j'+i0)%512
# and the core's own i-rows are always device columns 0..255. Host un-rotates.

import numpy as np

BS, N, EMB = 4, 512, 64
IN_DIM, HID = 256, 128
NCORES = 8
HALF = N // 2  # rows per core

_rows, _cols = np.nonzero(~np.eye(N, dtype=bool))

_COMPILED = {}


def _build_program():
    import concourse.tile as tile
    from concourse import bacc, mybir

    dt = mybir.dt
    f32 = dt.float32
    f32r = dt.float32r
    bf16 = dt.bfloat16
    AF = mybir.ActivationFunctionType
    OP = mybir.AluOpType

    nc = bacc.Bacc("TRN2", target_bir_lowering=False, debug=False)

    # ---- DRAM I/O (per-core shapes) ----
    nfT0 = nc.dram_tensor("nfT0", [128, N], f32, kind="ExternalInput").ap()
    nfT1 = nc.dram_tensor("nfT1", [128, N], f32, kind="ExternalInput").ap()
    fc1w0 = nc.dram_tensor("fc1w0", [128, HID], f32, kind="ExternalInput").ap()
    fc1w1 = nc.dram_tensor("fc1w1", [128, HID], f32, kind="ExternalInput").ap()
    fc2w = nc.dram_tensor("fc2w", [HID, EMB], f32, kind="ExternalInput").ap()
    w1 = nc.dram_tensor("w1", [EMB, EMB], f32, kind="ExternalInput").ap()
    w2 = nc.dram_tensor("w2", [EMB, EMB], f32, kind="ExternalInput").ap()
    # brow[0, 0:128]=fc1_b, [128:192]=fc2_b, [192:256]=e1_b
    brow = nc.dram_tensor("brow", [1, 256], f32, kind="ExternalInput").ap()
    # aux cols: 0/1 unused, 2=b_diff, 3=-b_diff (replicated down partitions)
    aux = nc.dram_tensor("aux", [128, 4], f32, kind="ExternalInput").ap()
    # aux2: [wd;0] and [0;wd] in bf16 (stationary side of the logit reduction)
    aux2 = nc.dram_tensor("aux2", [128, 2], bf16, kind="ExternalInput").ap()
    eye = nc.dram_tensor("eye", [128, 128], f32, kind="ExternalInput").ap()

    a_out = nc.dram_tensor("a_out", [HALF, N], f32, kind="ExternalOutput").ap()
    p0_out = nc.dram_tensor("p0_out", [HALF, N], f32, kind="ExternalOutput").ap()
    emb_out = nc.dram_tensor("emb_out", [HALF, EMB], f32, kind="ExternalOutput").ap()

    with tile.TileContext(nc) as tc:
        with (
            tc.tile_pool(name="const", bufs=1) as cpool,
            tc.tile_pool(name="wpair", bufs=4) as wpool,
            tc.tile_pool(name="eh", bufs=4) as ehpool,
            tc.tile_pool(name="sig", bufs=2) as sigpool,
            tc.tile_pool(name="chunk", bufs=2) as chpool,
            tc.tile_pool(name="psA", bufs=2, space="PSUM") as psA,
            tc.tile_pool(name="psL", bufs=2, space="PSUM") as psL,
            tc.tile_pool(name="psT", bufs=2, space="PSUM") as psT,
            tc.tile_pool(name="psP", bufs=2, space="PSUM") as psP,
        ):
            # ---- load constants ----
            t_nfT0 = cpool.tile([128, N], f32, tag="nfT0")
            t_nfT1 = cpool.tile([128, N], f32, tag="nfT1")
            t_fc1w0 = cpool.tile([128, HID], f32, tag="fc1w0")
            t_fc1w1 = cpool.tile([128, HID], f32, tag="fc1w1")
            t_fc2w = cpool.tile([HID, EMB], f32, tag="fc2w")
            t_w1 = cpool.tile([EMB, EMB], f32, tag="w1")
            t_w2 = cpool.tile([EMB, EMB], f32, tag="w2")
            t_brow = cpool.tile([1, 256], f32, tag="brow")
            t_aux = cpool.tile([128, 4], f32, tag="aux")
            t_aux2 = cpool.tile([128, 2], bf16, tag="aux2")
            t_eye = cpool.tile([128, 128], f32, tag="eye")
            nc.sync.dma_start(t_nfT0[:], nfT0)
            nc.sync.dma_start(t_nfT1[:], nfT1)
            nc.sync.dma_start(t_fc1w0[:], fc1w0)
            nc.sync.dma_start(t_fc1w1[:], fc1w1)
            nc.sync.dma_start(t_fc2w[:], fc2w)
            nc.sync.dma_start(t_w1[:], w1)
            nc.sync.dma_start(t_w2[:], w2)
            nc.sync.dma_start(t_brow[:], brow)
            nc.sync.dma_start(t_aux[:], aux)
            nc.sync.dma_start(t_aux2[:], aux2)
            nc.sync.dma_start(t_eye[:], eye)

            # In-place DVE self-copies: matmuls support only ONE semaphore
            # wait, so every DMA-fed matmul operand is re-written by the DVE
            # (program-ordered, one engine) before the PE touches it.
            for t in (t_nfT0, t_nfT1, t_fc1w0, t_fc1w1, t_fc2w, t_brow, t_eye, t_aux2):
                nc.vector.tensor_copy(t[:], t[:])

            t_ones = cpool.tile([1, N], f32, tag="ones")
            nc.vector.memset(t_ones[:], 1.0)
            t_w1r = cpool.tile([EMB, EMB], f32r, tag="w1r")
            nc.vector.tensor_copy(t_w1r[:], t_w1[:])

            def leaky(dst, src, scratch_pool):
                # max(x, 0.01x); only one PSUM operand allowed per instruction,
                # so stage 0.01x through SBUF first.
                t_s = scratch_pool.tile(
                    [src.partition_size(), src.free_size()], f32, tag="leak"
                )
                nc.vector.tensor_scalar_mul(t_s[:], src, 0.01)
                nc.vector.scalar_tensor_tensor(
                    dst, src, 0.0, t_s[:], op0=OP.add, op1=OP.max
                )

            # ---- fc head: hT = leaky(fc1_w.T @ nfT + fc1_b) ----
            ps_h = psP.tile([128, N], f32, tag="prep")
            nc.tensor.matmul(ps_h[:], t_fc1w0[:], t_nfT0[:], start=True, stop=False)
            nc.tensor.matmul(ps_h[:], t_fc1w1[:], t_nfT1[:], start=False, stop=False)
            nc.tensor.matmul(
                ps_h[:], t_brow[0:1, 0:128], t_ones[:], start=False, stop=True
            )
            t_hT = cpool.tile([128, N], f32, tag="hT")
            leaky(t_hT[:], ps_h[:], ehpool)

            # ---- embT = leaky(fc2_w.T @ hT + fc2_b), with zero columns at 512+ ----
            ps_e = psP.tile([EMB, N], f32, tag="prep")
            nc.tensor.matmul(ps_e[:], t_fc2w[:], t_hT[:], start=True, stop=False)
            nc.tensor.matmul(
                ps_e[:], t_brow[0:1, 128:192], t_ones[:], start=False, stop=True
            )
            t_embT = cpool.tile([EMB, N], f32r, tag="embT")
            leaky(t_embT[:], ps_e[:], ehpool)

            # ---- C2: relu-bias source. C2[0:64,c]=C(col c), C2[64:128,c]=C(col c+1)
            # where C(col) = W1.T @ embT[:,col] + e1_b ----
            ps_c = psP.tile([EMB, N], f32, tag="prep")
            nc.tensor.matmul(ps_c[:], t_w1r[:], t_embT[:], start=True, stop=False)
            nc.tensor.matmul(
                ps_c[:], t_brow[0:1, 192:256], t_ones[:], start=False, stop=True
            )
            t_Cs = cpool.tile([EMB, N], f32, tag="Cs")
            nc.scalar.copy(t_Cs[:], ps_c[:])
            t_C2 = cpool.tile([128, N], f32, tag="C2")
            nc.sync.dma_start(t_C2[0:EMB, :], t_Cs[:])
            nc.sync.dma_start(t_C2[EMB:128, 0 : N - 1], t_Cs[:, 1:N])

            # ---- emb output rows (this core's 256 rows = device cols 0..255) ----
            for k in range(2):
                ps_m = psP.tile([128, EMB], f32, tag="prep")
                nc.tensor.matmul(
                    ps_m[:],
                    t_hT[:, 128 * k : 128 * (k + 1)],
                    t_fc2w[:],
                    start=True,
                    stop=False,
                )
                nc.tensor.matmul(
                    ps_m[:],
                    t_ones[0:1, 0:128],
                    t_brow[0:1, 128:192],
                    start=False,
                    stop=True,
                )
                t_m = ehpool.tile([128, EMB], f32, tag="embrow")
                leaky(t_m[:], ps_m[:], ehpool)
                nc.sync.dma_start(emb_out[128 * k : 128 * (k + 1), :], t_m[:])

            # ---- main loop: 128 pairs of i-rows, in 2 groups of 64 ----
            NPAIR = HALF // 2
            GROUP = 64
            for g in range(NPAIR // GROUP):
                # logitsT bank: column 128*jb + 2*pp + s = logit(i_loc=2*pp+s) for
                # j rows 128*jb..128*jb+127
                ps_log = psL.tile([128, N], f32, tag="logT")
                for pp in range(GROUP):
                    p = g * GROUP + pp
                    iA = 2 * p  # device column of first row of the pair

                    t_w = wpool.tile([EMB, 128], f32r, tag="wpair")
                    build_eng = nc.vector
                    build_eng.scalar_tensor_tensor(
                        t_w[:, 0:EMB],
                        t_w2[:],
                        t_embT[:, iA : iA + 1],
                        t_w1[:],
                        op0=OP.mult,
                        op1=OP.subtract,
                    )
                    build_eng.scalar_tensor_tensor(
                        t_w[:, EMB:128],
                        t_w2[:],
                        t_embT[:, iA + 1 : iA + 2],
                        t_w1[:],
                        op0=OP.mult,
                        op1=OP.subtract,
                    )

                    ps_pre = psA.tile([128, N], f32, tag="pre")
                    nc.tensor.matmul(
                        ps_pre[:],
                        t_w[:],
                        t_embT[:, 0:N],
                    )

                    t_eh = ehpool.tile([128, N], bf16, tag="eh")
                    if (p % 32) < 25:
                        nc.scalar.activation(
                            t_eh[:],
                            ps_pre[:],
                            AF.Relu,
                            bias=t_C2[:, iA : iA + 1],
                            scale=1.0,
                        )
                    else:
                        nc.vector.tensor_scalar(
                            t_eh[:],
                            ps_pre[:],
                            t_C2[:, iA : iA + 1],
                            0.0,
                            op0=OP.add,
                            op1=OP.max,
                        )

                    for jb in range(4):
                        nc.tensor.matmul(
                            ps_log[:, 128 * jb + 2 * pp : 128 * jb + 2 * pp + 2],
                            t_eh[:, 128 * jb : 128 * (jb + 1)],
                            t_aux2[:],
                        )

                # batched sigmoid over the whole group (still j-major)
                t_p1T = sigpool.tile([128, N], f32, tag="p1T")
                t_p0T = sigpool.tile([128, N], f32, tag="p0T")
                nc.scalar.activation(
                    t_p1T[:], ps_log[:], AF.Sigmoid, bias=t_aux[:, 2:3], scale=1.0
                )
                nc.scalar.activation(
                    t_p0T[:], ps_log[:], AF.Sigmoid, bias=t_aux[:, 3:4], scale=-1.0
                )

                # transpose back to row-major [i_loc, j] chunks and ship out
                for s, (srcT, dram) in enumerate(((t_p0T, p0_out), (t_p1T, a_out))):
                    t_chunk = chpool.tile([128, N], f32, tag=f"chunk{s}")
                    for jb in range(4):
                        ps_t = psT.tile([128, 128], f32, tag="tr")
                        nc.tensor.transpose(
                            ps_t[:], srcT[:, 128 * jb : 128 * (jb + 1)], t_eye[:]
                        )
                        if s == 0:
                            nc.vector.tensor_copy(
                                t_chunk[:, 128 * jb : 128 * (jb + 1)], ps_t[:]
                            )
                        else:
                            nc.scalar.copy(
                                t_chunk[:, 128 * jb : 128 * (jb + 1)], ps_t[:]
                            )
                    nc.sync.dma_start(
                        dram[128 * g : 128 * (g + 1), :], t_chunk[:]
                    )

    nc.compile()
    return nc


def _get_program():
    if "nc" not in _COMPILED:
        _COMPILED["nc"] = _build_program()
    return _COMPILED["nc"]


def _make_in_maps(inputs):
    import ml_dtypes

    nf = np.asarray(inputs["node_features"], np.float32)
    fc1_w = np.asarray(inputs["fc1_w"], np.float32)
    fc1_b = np.asarray(inputs["fc1_b"], np.float32)
    fc2_w = np.asarray(inputs["fc2_w"], np.float32)
    fc2_b = np.asarray(inputs["fc2_b"], np.float32)
    e1_w = np.asarray(inputs["e1_w"], np.float32)
    e1_b = np.asarray(inputs["e1_b"], np.float32)
    e2_w = np.asarray(inputs["e2_w"], np.float32)
    e2_b = np.asarray(inputs["e2_b"], np.float32)

    wd = e2_w[:, 1] - e2_w[:, 0]  # [64]
    b_diff = float(e2_b[1] - e2_b[0])

    brow = np.zeros((1, 256), np.float32)
    brow[0, 0:128] = fc1_b
    brow[0, 128:192] = fc2_b
    brow[0, 192:256] = e1_b

    aux = np.zeros((128, 4), np.float32)
    aux[:, 2] = b_diff
    aux[:, 3] = -b_diff

    aux2 = np.zeros((128, 2), np.float32)
    aux2[0:64, 0] = wd
    aux2[64:128, 1] = wd
    aux2 = aux2.astype(ml_dtypes.bfloat16)

    common = {
        "fc1w0": np.ascontiguousarray(fc1_w[0:128]),
        "fc1w1": np.ascontiguousarray(fc1_w[128:256]),
        "fc2w": np.ascontiguousarray(fc2_w),
        "w1": np.ascontiguousarray(e1_w[0:64]),
        "w2": np.ascontiguousarray(e1_w[64:128]),
        "brow": brow,
        "aux": aux,
        "aux2": aux2,
        "eye": np.eye(128, dtype=np.float32),
    }

    in_maps = []
    for c in range(NCORES):
        b, i0 = c // 2, HALF * (c % 2)
        nfT = np.ascontiguousarray(nf[b].T)  # [256, 512]
        if i0:
            nfT = np.ascontiguousarray(np.roll(nfT, -i0, axis=1))
        m = dict(common)
        m["nfT0"] = np.ascontiguousarray(nfT[0:128])
        m["nfT1"] = np.ascontiguousarray(nfT[128:256])
        in_maps.append(m)
    return in_maps


def _assemble(results):
    adjacency = np.empty((BS, N, N), np.float32)
    p0 = np.empty((BS, N, N), np.float32)
    emb = np.empty((BS, N, EMB), np.float32)
    for c in range(NCORES):
        b, i0 = c // 2, HALF * (c % 2)
        ac = np.asarray(results[c]["a_out"])
        pc = np.asarray(results[c]["p0_out"])
        if i0:
            ac = np.roll(ac, i0, axis=1)
            pc = np.roll(pc, i0, axis=1)
        adjacency[b, i0 : i0 + HALF] = ac
        p0[b, i0 : i0 + HALF] = pc
        emb[b, i0 : i0 + HALF] = np.asarray(results[c]["emb_out"])
    pred = np.stack([p0[:, _rows, _cols], adjacency[:, _rows, _cols]], axis=-1)
    prediction = np.ascontiguousarray(pred.reshape(BS, -1))
    return adjacency, prediction, emb


def kernel(**inputs):
    from concourse import bass_utils

    nc = _get_program()
    in_maps = _make_in_maps(inputs)
    res = bass_utils.run_bass_kernel_spmd(nc, in_maps, core_ids=list(range(NCORES)))
    return _assemble(res.results)


# revision 11
# speedup vs baseline: 1.0815x; 1.0815x over previous
# Trainium2 Bass kernel for FC_STGNN pedestrian edge-MLP (gnn_message_passing).
#
# Reference computation (BS=4, N=512, EMB=64):
#   h    = leaky(nf @ fc1_w + fc1_b)            [4,512,128]
#   emb  = leaky(h @ fc2_w + fc2_b)             [4,512,64]
#   edge_in[b,i,j] = [emb_i - emb_j, emb_i*emb_j]
#   eh   = relu(edge_in @ e1_w + e1_b)          [4,512,512,64]
#   logits = eh @ e2_w + e2_b                   [4,512,512,2]
#   edge_prob = softmax(logits)                 -> adjacency = p1, prediction = off-diag (p0,p1)
#
# Kernel restructuring:
#   * softmax over 2 classes == sigmoid of the logit difference:
#       d = eh @ (e2_w[:,1]-e2_w[:,0]) + (e2_b[1]-e2_b[0]);  p1 = sigmoid(d), p0 = sigmoid(-d)
#   * pre-activation factorization (W1 = e1_w[:64], W2 = e1_w[64:]):
#       pre[i,j,:] = emb_j @ (diag(emb_i) @ W2 - W1) + (emb_i @ W1 + e1_b)
#     For each pair of output rows (i, i+1) we build a [64,128] weight (two fused
#     vector ops), run one [64x128x512] fp32r matmul against the shared embT, and
#     absorb the per-i constant as the relu's per-partition bias (C2 trick below).
#   * the logit reduction runs with eh as the (bf16, FWL) stationary operand so the
#     [2,512] per-pair logits land as columns of a shared PSUM bank; sigmoid is then
#     batched over 64 pairs and PE-transposed back to row-major output chunks.
#
# Sharding: 8 cores; core c -> batch b=c//2, i-rows [i0, i0+256) with i0=256*(c%2).
# The per-core program is identical: the j axis is ROTATED by -i0 on the host
# (nfT input is np.roll'ed), so device columns j' correspond to real j=(j'+i0)%512
# and the core's own i-rows are always device columns 0..255. Host un-rotates.

import numpy as np

BS, N, EMB = 4, 512, 64
IN_DIM, HID = 256, 128
NCORES = 8
HALF = N // 2  # rows per core

_rows, _cols = np.nonzero(~np.eye(N, dtype=bool))

_COMPILED = {}


def _build_program():
    import concourse.tile as tile
    from concourse import bacc, mybir

    dt = mybir.dt
    f32 = dt.float32
    f32r = dt.float32r
    bf16 = dt.bfloat16
    AF = mybir.ActivationFunctionType
    OP = mybir.AluOpType

    nc = bacc.Bacc("TRN2", target_bir_lowering=False, debug=False)

    # ---- DRAM I/O (per-core shapes) ----
    nfT0 = nc.dram_tensor("nfT0", [128, N], f32, kind="ExternalInput").ap()
    nfT1 = nc.dram_tensor("nfT1", [128, N], f32, kind="ExternalInput").ap()
    fc1w0 = nc.dram_tensor("fc1w0", [128, HID], f32, kind="ExternalInput").ap()
    fc1w1 = nc.dram_tensor("fc1w1", [128, HID], f32, kind="ExternalInput").ap()
    fc2w = nc.dram_tensor("fc2w", [HID, EMB], f32, kind="ExternalInput").ap()
    w1 = nc.dram_tensor("w1", [EMB, EMB], f32, kind="ExternalInput").ap()
    w2 = nc.dram_tensor("w2", [EMB, EMB], f32, kind="ExternalInput").ap()
    # brow[0, 0:128]=fc1_b, [128:192]=fc2_b, [192:256]=e1_b
    brow = nc.dram_tensor("brow", [1, 256], f32, kind="ExternalInput").ap()
    # aux cols: 0/1 unused, 2=b_diff, 3=-b_diff (replicated down partitions)
    aux = nc.dram_tensor("aux", [128, 4], f32, kind="ExternalInput").ap()
    # aux2: [wd;0] and [0;wd] in bf16 (stationary side of the logit reduction)
    aux2 = nc.dram_tensor("aux2", [128, 2], bf16, kind="ExternalInput").ap()
    eye = nc.dram_tensor("eye", [128, 128], f32, kind="ExternalInput").ap()

    a_out = nc.dram_tensor("a_out", [HALF, N], f32, kind="ExternalOutput").ap()
    p0_out = nc.dram_tensor("p0_out", [HALF, N], f32, kind="ExternalOutput").ap()
    emb_out = nc.dram_tensor("emb_out", [HALF, EMB], f32, kind="ExternalOutput").ap()

    with tile.TileContext(nc) as tc:
        with (
            tc.tile_pool(name="const", bufs=1) as cpool,
            tc.tile_pool(name="wpair", bufs=6) as wpool,
            tc.tile_pool(name="eh", bufs=6) as ehpool,
            tc.tile_pool(name="sig", bufs=2) as sigpool,
            tc.tile_pool(name="chunk", bufs=2) as chpool,
            tc.tile_pool(name="psA", bufs=2, space="PSUM") as psA,
            tc.tile_pool(name="psL", bufs=2, space="PSUM") as psL,
            tc.tile_pool(name="psT", bufs=2, space="PSUM") as psT,
            tc.tile_pool(name="psP", bufs=2, space="PSUM") as psP,
        ):
            # ---- load constants ----
            t_nfT0 = cpool.tile([128, N], f32, tag="nfT0")
            t_nfT1 = cpool.tile([128, N], f32, tag="nfT1")
            t_fc1w0 = cpool.tile([128, HID], f32, tag="fc1w0")
            t_fc1w1 = cpool.tile([128, HID], f32, tag="fc1w1")
            t_fc2w = cpool.tile([HID, EMB], f32, tag="fc2w")
            t_w1 = cpool.tile([EMB, EMB], f32, tag="w1")
            t_w2 = cpool.tile([EMB, EMB], f32, tag="w2")
            t_brow = cpool.tile([1, 256], f32, tag="brow")
            t_aux = cpool.tile([128, 4], f32, tag="aux")
            t_aux2 = cpool.tile([128, 2], bf16, tag="aux2")
            t_eye = cpool.tile([128, 128], f32, tag="eye")
            nc.sync.dma_start(t_nfT0[:], nfT0)
            nc.sync.dma_start(t_nfT1[:], nfT1)
            nc.sync.dma_start(t_fc1w0[:], fc1w0)
            nc.sync.dma_start(t_fc1w1[:], fc1w1)
            nc.sync.dma_start(t_fc2w[:], fc2w)
            nc.sync.dma_start(t_w1[:], w1)
            nc.sync.dma_start(t_w2[:], w2)
            nc.sync.dma_start(t_brow[:], brow)
            nc.sync.dma_start(t_aux[:], aux)
            nc.sync.dma_start(t_aux2[:], aux2)
            nc.sync.dma_start(t_eye[:], eye)

            # In-place DVE self-copies: matmuls support only ONE semaphore
            # wait, so every DMA-fed matmul operand is re-written by the DVE
            # (program-ordered, one engine) before the PE touches it.
            for t in (t_nfT0, t_nfT1, t_fc1w0, t_fc1w1, t_fc2w, t_brow, t_eye, t_aux2):
                nc.vector.tensor_copy(t[:], t[:])

            t_ones = cpool.tile([1, N], f32, tag="ones")
            nc.vector.memset(t_ones[:], 1.0)
            t_w1r = cpool.tile([EMB, EMB], f32r, tag="w1r")
            nc.vector.tensor_copy(t_w1r[:], t_w1[:])

            def leaky(dst, src, scratch_pool):
                # max(x, 0.01x); only one PSUM operand allowed per instruction,
                # so stage 0.01x through SBUF first.
                t_s = scratch_pool.tile(
                    [src.partition_size(), src.free_size()], f32, tag="leak"
                )
                nc.vector.tensor_scalar_mul(t_s[:], src, 0.01)
                nc.vector.scalar_tensor_tensor(
                    dst, src, 0.0, t_s[:], op0=OP.add, op1=OP.max
                )

            # ---- fc head: hT = leaky(fc1_w.T @ nfT + fc1_b) ----
            ps_h = psP.tile([128, N], f32, tag="prep")
            nc.tensor.matmul(ps_h[:], t_fc1w0[:], t_nfT0[:], start=True, stop=False)
            nc.tensor.matmul(ps_h[:], t_fc1w1[:], t_nfT1[:], start=False, stop=False)
            nc.tensor.matmul(
                ps_h[:], t_brow[0:1, 0:128], t_ones[:], start=False, stop=True
            )
            t_hT = cpool.tile([128, N], f32, tag="hT")
            leaky(t_hT[:], ps_h[:], ehpool)

            # ---- embT = leaky(fc2_w.T @ hT + fc2_b), with zero columns at 512+ ----
            ps_e = psP.tile([EMB, N], f32, tag="prep")
            nc.tensor.matmul(ps_e[:], t_fc2w[:], t_hT[:], start=True, stop=False)
            nc.tensor.matmul(
                ps_e[:], t_brow[0:1, 128:192], t_ones[:], start=False, stop=True
            )
            t_embT = cpool.tile([EMB, N], f32r, tag="embT")
            leaky(t_embT[:], ps_e[:], ehpool)

            # ---- C2: relu-bias source. C2[0:64,c]=C(col c), C2[64:128,c]=C(col c+1)
            # where C(col) = W1.T @ embT[:,col] + e1_b ----
            ps_c = psP.tile([EMB, N], f32, tag="prep")
            nc.tensor.matmul(ps_c[:], t_w1r[:], t_embT[:], start=True, stop=False)
            nc.tensor.matmul(
                ps_c[:], t_brow[0:1, 192:256], t_ones[:], start=False, stop=True
            )
            t_Cs = cpool.tile([EMB, N], f32, tag="Cs")
            nc.scalar.copy(t_Cs[:], ps_c[:])
            t_C2 = cpool.tile([128, N], f32, tag="C2")
            nc.sync.dma_start(t_C2[0:EMB, :], t_Cs[:])
            nc.sync.dma_start(t_C2[EMB:128, 0 : N - 1], t_Cs[:, 1:N])

            # ---- emb output rows (this core's 256 rows = device cols 0..255) ----
            for k in range(2):
                ps_m = psP.tile([128, EMB], f32, tag="prep")
                nc.tensor.matmul(
                    ps_m[:],
                    t_hT[:, 128 * k : 128 * (k + 1)],
                    t_fc2w[:],
                    start=True,
                    stop=False,
                )
                nc.tensor.matmul(
                    ps_m[:],
                    t_ones[0:1, 0:128],
                    t_brow[0:1, 128:192],
                    start=False,
                    stop=True,
                )
                t_m = ehpool.tile([128, EMB], f32, tag="embrow")
                leaky(t_m[:], ps_m[:], ehpool)
                nc.sync.dma_start(emb_out[128 * k : 128 * (k + 1), :], t_m[:])

            # ---- main loop: 128 pairs of i-rows, in 2 groups of 64 ----
            NPAIR = HALF // 2
            GROUP = 64
            for g in range(NPAIR // GROUP):
                # logitsT bank: column 128*jb + 2*pp + s = logit(i_loc=2*pp+s) for
                # j rows 128*jb..128*jb+127
                ps_log = psL.tile([128, N], f32, tag="logT")
                for pp in range(GROUP):
                    p = g * GROUP + pp
                    iA = 2 * p  # device column of first row of the pair

                    t_w = wpool.tile([EMB, 128], f32r, tag="wpair")
                    if p % 16 < 7:
                        # Pool engine path: mult-with-broadcast + subtract
                        t_tmp = wpool.tile([EMB, 128], f32, tag="wtmp")
                        for q in range(2):
                            bc = t_embT[:, iA + q : iA + q + 1].broadcast_to([EMB, EMB])
                            nc.gpsimd.tensor_tensor(
                                t_tmp[:, EMB * q : EMB * (q + 1)],
                                t_w2[:],
                                bc,
                                op=OP.mult,
                            )
                            nc.gpsimd.tensor_tensor(
                                t_w[:, EMB * q : EMB * (q + 1)],
                                t_tmp[:, EMB * q : EMB * (q + 1)],
                                t_w1[:],
                                op=OP.subtract,
                            )
                    else:
                        for q in range(2):
                            nc.vector.scalar_tensor_tensor(
                                t_w[:, EMB * q : EMB * (q + 1)],
                                t_w2[:],
                                t_embT[:, iA + q : iA + q + 1],
                                t_w1[:],
                                op0=OP.mult,
                                op1=OP.subtract,
                            )

                    ps_pre = psA.tile([128, N], f32, tag="pre")
                    nc.tensor.matmul(
                        ps_pre[:],
                        t_w[:],
                        t_embT[:, 0:N],
                    )

                    t_eh = ehpool.tile([128, N], bf16, tag="eh")
                    if p % 16 not in (7, 11, 15):
                        nc.scalar.activation(
                            t_eh[:],
                            ps_pre[:],
                            AF.Relu,
                            bias=t_C2[:, iA : iA + 1],
                            scale=1.0,
                        )
                    else:
                        nc.vector.tensor_scalar(
                            t_eh[:],
                            ps_pre[:],
                            t_C2[:, iA : iA + 1],
                            0.0,
                            op0=OP.add,
                            op1=OP.max,
                        )

                    for jb in range(4):
                        nc.tensor.matmul(
                            ps_log[:, 128 * jb + 2 * pp : 128 * jb + 2 * pp + 2],
                            t_eh[:, 128 * jb : 128 * (jb + 1)],
                            t_aux2[:],
                        )

                # batched sigmoid over the whole group (still j-major)
                t_p1T = sigpool.tile([128, N], f32, tag="p1T")
                t_p0T = sigpool.tile([128, N], f32, tag="p0T")
                nc.scalar.activation(
                    t_p1T[:], ps_log[:], AF.Sigmoid, bias=t_aux[:, 2:3], scale=1.0
                )
                nc.scalar.activation(
                    t_p0T[:], ps_log[:], AF.Sigmoid, bias=t_aux[:, 3:4], scale=-1.0
                )

                # transpose back to row-major [i_loc, j] chunks and ship out
                for s, (srcT, dram) in enumerate(((t_p0T, p0_out), (t_p1T, a_out))):
                    t_chunk = chpool.tile([128, N], f32, tag=f"chunk{s}")
                    for jb in range(4):
                        ps_t = psT.tile([128, 128], f32, tag="tr")
                        nc.tensor.transpose(
                            ps_t[:], srcT[:, 128 * jb : 128 * (jb + 1)], t_eye[:]
                        )
                        if s == 0:
                            nc.vector.tensor_copy(
                                t_chunk[:, 128 * jb : 128 * (jb + 1)], ps_t[:]
                            )
                        else:
                            nc.scalar.copy(
                                t_chunk[:, 128 * jb : 128 * (jb + 1)], ps_t[:]
                            )
                    nc.sync.dma_start(
                        dram[128 * g : 128 * (g + 1), :], t_chunk[:]
                    )

    nc.compile()
    return nc


def _get_program():
    if "nc" not in _COMPILED:
        _COMPILED["nc"] = _build_program()
    return _COMPILED["nc"]


def _make_in_maps(inputs):
    import ml_dtypes

    nf = np.asarray(inputs["node_features"], np.float32)
    fc1_w = np.asarray(inputs["fc1_w"], np.float32)
    fc1_b = np.asarray(inputs["fc1_b"], np.float32)
    fc2_w = np.asarray(inputs["fc2_w"], np.float32)
    fc2_b = np.asarray(inputs["fc2_b"], np.float32)
    e1_w = np.asarray(inputs["e1_w"], np.float32)
    e1_b = np.asarray(inputs["e1_b"], np.float32)
    e2_w = np.asarray(inputs["e2_w"], np.float32)
    e2_b = np.asarray(inputs["e2_b"], np.float32)

    wd = e2_w[:, 1] - e2_w[:, 0]  # [64]
    b_diff = float(e2_b[1] - e2_b[0])

    brow = np.zeros((1, 256), np.float32)
    brow[0, 0:128] = fc1_b
    brow[0, 128:192] = fc2_b
    brow[0, 192:256] = e1_b

    aux = np.zeros((128, 4), np.float32)
    aux[:, 2] = b_diff
    aux[:, 3] = -b_diff

    aux2 = np.zeros((128, 2), np.float32)
    aux2[0:64, 0] = wd
    aux2[64:128, 1] = wd
    aux2 = aux2.astype(ml_dtypes.bfloat16)

    common = {
        "fc1w0": np.ascontiguousarray(fc1_w[0:128]),
        "fc1w1": np.ascontiguousarray(fc1_w[128:256]),
        "fc2w": np.ascontiguousarray(fc2_w),
        "w1": np.ascontiguousarray(e1_w[0:64]),
        "w2": np.ascontiguousarray(e1_w[64:128]),
        "brow": brow,
        "aux": aux,
        "aux2": aux2,
        "eye": np.eye(128, dtype=np.float32),
    }

    in_maps = []
    for c in range(NCORES):
        b, i0 = c // 2, HALF * (c % 2)
        nfT = np.ascontiguousarray(nf[b].T)  # [256, 512]
        if i0:
            nfT = np.ascontiguousarray(np.roll(nfT, -i0, axis=1))
        m = dict(common)
        m["nfT0"] = np.ascontiguousarray(nfT[0:128])
        m["nfT1"] = np.ascontiguousarray(nfT[128:256])
        in_maps.append(m)
    return in_maps


def _assemble(results):
    adjacency = np.empty((BS, N, N), np.float32)
    p0 = np.empty((BS, N, N), np.float32)
    emb = np.empty((BS, N, EMB), np.float32)
    for c in range(NCORES):
        b, i0 = c // 2, HALF * (c % 2)
        ac = np.asarray(results[c]["a_out"])
        pc = np.asarray(results[c]["p0_out"])
        if i0:
            ac = np.roll(ac, i0, axis=1)
            pc = np.roll(pc, i0, axis=1)
        adjacency[b, i0 : i0 + HALF] = ac
        p0[b, i0 : i0 + HALF] = pc
        emb[b, i0 : i0 + HALF] = np.asarray(results[c]["emb_out"])
    pred = np.stack([p0[:, _rows, _cols], adjacency[:, _rows, _cols]], axis=-1)
    prediction = np.ascontiguousarray(pred.reshape(BS, -1))
    return adjacency, prediction, emb


def kernel(**inputs):
    from concourse import bass_utils

    nc = _get_program()
    in_maps = _make_in_maps(inputs)
    res = bass_utils.run_bass_kernel_spmd(nc, in_maps, core_ids=list(range(NCORES)))
    return _assemble(res.results)


# revision 12
# speedup vs baseline: 1.1629x; 1.0753x over previous
# Trainium2 Bass kernel for FC_STGNN pedestrian edge-MLP (gnn_message_passing).
#
# Reference computation (BS=4, N=512, EMB=64):
#   h    = leaky(nf @ fc1_w + fc1_b)            [4,512,128]
#   emb  = leaky(h @ fc2_w + fc2_b)             [4,512,64]
#   edge_in[b,i,j] = [emb_i - emb_j, emb_i*emb_j]
#   eh   = relu(edge_in @ e1_w + e1_b)          [4,512,512,64]
#   logits = eh @ e2_w + e2_b                   [4,512,512,2]
#   edge_prob = softmax(logits)                 -> adjacency = p1, prediction = off-diag (p0,p1)
#
# Kernel restructuring:
#   * softmax over 2 classes == sigmoid of the logit difference:
#       d = eh @ (e2_w[:,1]-e2_w[:,0]) + (e2_b[1]-e2_b[0]);  p1 = sigmoid(d), p0 = sigmoid(-d)
#   * pre-activation factorization (W1 = e1_w[:64], W2 = e1_w[64:]):
#       pre[i,j,:] = emb_j @ (diag(emb_i) @ W2 - W1) + (emb_i @ W1 + e1_b)
#     For each pair of output rows (i, i+1) we build a [64,128] weight (two fused
#     vector ops), run one [64x128x512] fp32r matmul against the shared embT, and
#     absorb the per-i constant as the relu's per-partition bias (C2 trick below).
#   * the logit reduction runs with eh as the (bf16, FWL) stationary operand so the
#     [2,512] per-pair logits land as columns of a shared PSUM bank; sigmoid is then
#     batched over 64 pairs and PE-transposed back to row-major output chunks.
#
# Sharding: 8 cores; core c -> batch b=c//2, i-rows [i0, i0+256) with i0=256*(c%2).
# The per-core program is identical: the j axis is ROTATED by -i0 on the host
# (nfT input is np.roll'ed), so device columns j' correspond to real j=(j'+i0)%512
# and the core's own i-rows are always device columns 0..255. Host un-rotates.

import numpy as np

BS, N, EMB = 4, 512, 64
IN_DIM, HID = 256, 128
NCORES = 8
HALF = N // 2  # rows per core

_rows, _cols = np.nonzero(~np.eye(N, dtype=bool))

_COMPILED = {}


def _build_program():
    import concourse.tile as tile
    from concourse import bacc, mybir

    dt = mybir.dt
    f32 = dt.float32
    f32r = dt.float32r
    bf16 = dt.bfloat16
    AF = mybir.ActivationFunctionType
    OP = mybir.AluOpType

    nc = bacc.Bacc("TRN2", target_bir_lowering=False, debug=False)

    # ---- DRAM I/O (per-core shapes) ----
    nfT0 = nc.dram_tensor("nfT0", [128, N], f32, kind="ExternalInput").ap()
    nfT1 = nc.dram_tensor("nfT1", [128, N], f32, kind="ExternalInput").ap()
    fc1w0 = nc.dram_tensor("fc1w0", [128, HID], f32, kind="ExternalInput").ap()
    fc1w1 = nc.dram_tensor("fc1w1", [128, HID], f32, kind="ExternalInput").ap()
    fc2w = nc.dram_tensor("fc2w", [HID, EMB], f32, kind="ExternalInput").ap()
    w1 = nc.dram_tensor("w1", [EMB, EMB], f32, kind="ExternalInput").ap()
    w2 = nc.dram_tensor("w2", [EMB, EMB], f32, kind="ExternalInput").ap()
    # brow[0, 0:128]=fc1_b, [128:192]=fc2_b, [192:256]=e1_b
    brow = nc.dram_tensor("brow", [1, 256], f32, kind="ExternalInput").ap()
    # aux cols: 0/1 unused, 2=b_diff, 3=-b_diff (replicated down partitions)
    aux = nc.dram_tensor("aux", [128, 4], f32, kind="ExternalInput").ap()
    # aux2: [wd;0] and [0;wd] in bf16 (stationary side of the logit reduction)
    aux2 = nc.dram_tensor("aux2", [128, 2], bf16, kind="ExternalInput").ap()
    eye = nc.dram_tensor("eye", [128, 128], f32, kind="ExternalInput").ap()

    a_out = nc.dram_tensor("a_out", [HALF, N], f32, kind="ExternalOutput").ap()
    p0_out = nc.dram_tensor("p0_out", [HALF, N], f32, kind="ExternalOutput").ap()
    emb_out = nc.dram_tensor("emb_out", [HALF, EMB], f32, kind="ExternalOutput").ap()

    with tile.TileContext(nc) as tc:
        with (
            tc.tile_pool(name="const", bufs=1) as cpool,
            tc.tile_pool(name="wpair", bufs=6) as wpool,
            tc.tile_pool(name="eh", bufs=6) as ehpool,
            tc.tile_pool(name="sig", bufs=2) as sigpool,
            tc.tile_pool(name="chunk", bufs=2) as chpool,
            tc.tile_pool(name="psA", bufs=2, space="PSUM") as psA,
            tc.tile_pool(name="psL", bufs=2, space="PSUM") as psL,
            tc.tile_pool(name="psT", bufs=2, space="PSUM") as psT,
            tc.tile_pool(name="psP", bufs=2, space="PSUM") as psP,
        ):
            # ---- load constants ----
            t_nfT0 = cpool.tile([128, N], f32, tag="nfT0")
            t_nfT1 = cpool.tile([128, N], f32, tag="nfT1")
            t_fc1w0 = cpool.tile([128, HID], f32, tag="fc1w0")
            t_fc1w1 = cpool.tile([128, HID], f32, tag="fc1w1")
            t_fc2w = cpool.tile([HID, EMB], f32, tag="fc2w")
            t_w1 = cpool.tile([EMB, EMB], f32, tag="w1")
            t_w2 = cpool.tile([EMB, EMB], f32, tag="w2")
            t_brow = cpool.tile([1, 256], f32, tag="brow")
            t_aux = cpool.tile([128, 4], f32, tag="aux")
            t_aux2 = cpool.tile([128, 2], bf16, tag="aux2")
            t_eye = cpool.tile([128, 128], f32, tag="eye")
            nc.sync.dma_start(t_nfT0[:], nfT0)
            nc.sync.dma_start(t_nfT1[:], nfT1)
            nc.sync.dma_start(t_fc1w0[:], fc1w0)
            nc.sync.dma_start(t_fc1w1[:], fc1w1)
            nc.sync.dma_start(t_fc2w[:], fc2w)
            nc.sync.dma_start(t_w1[:], w1)
            nc.sync.dma_start(t_w2[:], w2)
            nc.sync.dma_start(t_brow[:], brow)
            nc.sync.dma_start(t_aux[:], aux)
            nc.sync.dma_start(t_aux2[:], aux2)
            nc.sync.dma_start(t_eye[:], eye)

            # In-place DVE self-copies: matmuls support only ONE semaphore
            # wait, so every DMA-fed matmul operand is re-written by the DVE
            # (program-ordered, one engine) before the PE touches it.
            for t in (t_nfT0, t_nfT1, t_fc1w0, t_fc1w1, t_fc2w, t_brow, t_eye, t_aux2):
                nc.vector.tensor_copy(t[:], t[:])

            t_ones = cpool.tile([1, N], f32, tag="ones")
            nc.vector.memset(t_ones[:], 1.0)
            t_w1r = cpool.tile([EMB, EMB], f32r, tag="w1r")
            nc.vector.tensor_copy(t_w1r[:], t_w1[:])

            def leaky(dst, src, scratch_pool):
                # max(x, 0.01x); only one PSUM operand allowed per instruction,
                # so stage 0.01x through SBUF first.
                t_s = scratch_pool.tile(
                    [src.partition_size(), src.free_size()], f32, tag="leak"
                )
                nc.vector.tensor_scalar_mul(t_s[:], src, 0.01)
                nc.vector.scalar_tensor_tensor(
                    dst, src, 0.0, t_s[:], op0=OP.add, op1=OP.max
                )

            # ---- fc head: hT = leaky(fc1_w.T @ nfT + fc1_b) ----
            ps_h = psP.tile([128, N], f32, tag="prep")
            nc.tensor.matmul(ps_h[:], t_fc1w0[:], t_nfT0[:], start=True, stop=False)
            nc.tensor.matmul(ps_h[:], t_fc1w1[:], t_nfT1[:], start=False, stop=False)
            nc.tensor.matmul(
                ps_h[:], t_brow[0:1, 0:128], t_ones[:], start=False, stop=True
            )
            t_hT = cpool.tile([128, N], f32, tag="hT")
            leaky(t_hT[:], ps_h[:], ehpool)

            # ---- embT = leaky(fc2_w.T @ hT + fc2_b), with zero columns at 512+ ----
            ps_e = psP.tile([EMB, N], f32, tag="prep")
            nc.tensor.matmul(ps_e[:], t_fc2w[:], t_hT[:], start=True, stop=False)
            nc.tensor.matmul(
                ps_e[:], t_brow[0:1, 128:192], t_ones[:], start=False, stop=True
            )
            t_embT = cpool.tile([EMB, N], f32r, tag="embT")
            leaky(t_embT[:], ps_e[:], ehpool)

            # ---- C2: relu-bias source. C2[0:64,c]=C(col c), C2[64:128,c]=C(col c+1)
            # where C(col) = W1.T @ embT[:,col] + e1_b ----
            ps_c = psP.tile([EMB, N], f32, tag="prep")
            nc.tensor.matmul(ps_c[:], t_w1r[:], t_embT[:], start=True, stop=False)
            nc.tensor.matmul(
                ps_c[:], t_brow[0:1, 192:256], t_ones[:], start=False, stop=True
            )
            t_Cs = cpool.tile([EMB, N], f32, tag="Cs")
            nc.scalar.copy(t_Cs[:], ps_c[:])
            t_C2 = cpool.tile([128, N], f32, tag="C2")
            nc.sync.dma_start(t_C2[0:EMB, :], t_Cs[:])
            nc.sync.dma_start(t_C2[EMB:128, 0 : N - 1], t_Cs[:, 1:N])

            # ---- emb output rows (this core's 256 rows = device cols 0..255) ----
            for k in range(2):
                ps_m = psP.tile([128, EMB], f32, tag="prep")
                nc.tensor.matmul(
                    ps_m[:],
                    t_hT[:, 128 * k : 128 * (k + 1)],
                    t_fc2w[:],
                    start=True,
                    stop=False,
                )
                nc.tensor.matmul(
                    ps_m[:],
                    t_ones[0:1, 0:128],
                    t_brow[0:1, 128:192],
                    start=False,
                    stop=True,
                )
                t_m = ehpool.tile([128, EMB], f32, tag="embrow")
                leaky(t_m[:], ps_m[:], ehpool)
                nc.sync.dma_start(emb_out[128 * k : 128 * (k + 1), :], t_m[:])

            # ---- main loop: 128 pairs of i-rows, in 2 groups of 64 ----
            NPAIR = HALF // 2
            GROUP = 64
            for g in range(NPAIR // GROUP):
                # logitsT bank: column 128*jb + 2*pp + s = logit(i_loc=2*pp+s) for
                # j rows 128*jb..128*jb+127
                ps_log = psL.tile([128, N], f32, tag="logT")
                for pp in range(GROUP):
                    p = g * GROUP + pp
                    iA = 2 * p  # device column of first row of the pair

                    t_w = wpool.tile([EMB, 128], f32r, tag="wpair")
                    if True:
                        # Pool engine path: mult-with-broadcast + subtract
                        t_tmp = wpool.tile([EMB, 128], f32, tag="wtmp")
                        for q in range(2):
                            bc = t_embT[:, iA + q : iA + q + 1].broadcast_to([EMB, EMB])
                            nc.gpsimd.tensor_tensor(
                                t_tmp[:, EMB * q : EMB * (q + 1)],
                                t_w2[:],
                                bc,
                                op=OP.mult,
                            )
                            nc.gpsimd.tensor_tensor(
                                t_w[:, EMB * q : EMB * (q + 1)],
                                t_tmp[:, EMB * q : EMB * (q + 1)],
                                t_w1[:],
                                op=OP.subtract,
                            )
                    else:
                        for q in range(2):
                            nc.vector.scalar_tensor_tensor(
                                t_w[:, EMB * q : EMB * (q + 1)],
                                t_w2[:],
                                t_embT[:, iA + q : iA + q + 1],
                                t_w1[:],
                                op0=OP.mult,
                                op1=OP.subtract,
                            )

                    ps_pre = psA.tile([128, N], f32, tag="pre")
                    nc.tensor.matmul(
                        ps_pre[:],
                        t_w[:],
                        t_embT[:, 0:N],
                    )

                    t_eh = ehpool.tile([128, N], bf16, tag="eh")
                    if p % 2 == 0:
                        nc.scalar.activation(
                            t_eh[:],
                            ps_pre[:],
                            AF.Relu,
                            bias=t_C2[:, iA : iA + 1],
                            scale=1.0,
                        )
                    else:
                        nc.vector.tensor_scalar(
                            t_eh[:],
                            ps_pre[:],
                            t_C2[:, iA : iA + 1],
                            0.0,
                            op0=OP.add,
                            op1=OP.max,
                        )

                    for jb in range(4):
                        nc.tensor.matmul(
                            ps_log[:, 128 * jb + 2 * pp : 128 * jb + 2 * pp + 2],
                            t_eh[:, 128 * jb : 128 * (jb + 1)],
                            t_aux2[:],
                        )

                # batched sigmoid over the whole group (still j-major)
                t_p1T = sigpool.tile([128, N], f32, tag="p1T")
                t_p0T = sigpool.tile([128, N], f32, tag="p0T")
                nc.scalar.activation(
                    t_p1T[:], ps_log[:], AF.Sigmoid, bias=t_aux[:, 2:3], scale=1.0
                )
                nc.scalar.activation(
                    t_p0T[:], ps_log[:], AF.Sigmoid, bias=t_aux[:, 3:4], scale=-1.0
                )

                # transpose back to row-major [i_loc, j] chunks and ship out
                for s, (srcT, dram) in enumerate(((t_p0T, p0_out), (t_p1T, a_out))):
                    t_chunk = chpool.tile([128, N], f32, tag=f"chunk{s}")
                    for jb in range(4):
                        ps_t = psT.tile([128, 128], f32, tag="tr")
                        nc.tensor.transpose(
                            ps_t[:], srcT[:, 128 * jb : 128 * (jb + 1)], t_eye[:]
                        )
                        if s == 0:
                            nc.vector.tensor_copy(
                                t_chunk[:, 128 * jb : 128 * (jb + 1)], ps_t[:]
                            )
                        else:
                            nc.scalar.copy(
                                t_chunk[:, 128 * jb : 128 * (jb + 1)], ps_t[:]
                            )
                    nc.sync.dma_start(
                        dram[128 * g : 128 * (g + 1), :], t_chunk[:]
                    )

    nc.compile()
    return nc


def _get_program():
    if "nc" not in _COMPILED:
        _COMPILED["nc"] = _build_program()
    return _COMPILED["nc"]


def _make_in_maps(inputs):
    import ml_dtypes

    nf = np.asarray(inputs["node_features"], np.float32)
    fc1_w = np.asarray(inputs["fc1_w"], np.float32)
    fc1_b = np.asarray(inputs["fc1_b"], np.float32)
    fc2_w = np.asarray(inputs["fc2_w"], np.float32)
    fc2_b = np.asarray(inputs["fc2_b"], np.float32)
    e1_w = np.asarray(inputs["e1_w"], np.float32)
    e1_b = np.asarray(inputs["e1_b"], np.float32)
    e2_w = np.asarray(inputs["e2_w"], np.float32)
    e2_b = np.asarray(inputs["e2_b"], np.float32)

    wd = e2_w[:, 1] - e2_w[:, 0]  # [64]
    b_diff = float(e2_b[1] - e2_b[0])

    brow = np.zeros((1, 256), np.float32)
    brow[0, 0:128] = fc1_b
    brow[0, 128:192] = fc2_b
    brow[0, 192:256] = e1_b

    aux = np.zeros((128, 4), np.float32)
    aux[:, 2] = b_diff
    aux[:, 3] = -b_diff

    aux2 = np.zeros((128, 2), np.float32)
    aux2[0:64, 0] = wd
    aux2[64:128, 1] = wd
    aux2 = aux2.astype(ml_dtypes.bfloat16)

    common = {
        "fc1w0": np.ascontiguousarray(fc1_w[0:128]),
        "fc1w1": np.ascontiguousarray(fc1_w[128:256]),
        "fc2w": np.ascontiguousarray(fc2_w),
        "w1": np.ascontiguousarray(e1_w[0:64]),
        "w2": np.ascontiguousarray(e1_w[64:128]),
        "brow": brow,
        "aux": aux,
        "aux2": aux2,
        "eye": np.eye(128, dtype=np.float32),
    }

    in_maps = []
    for c in range(NCORES):
        b, i0 = c // 2, HALF * (c % 2)
        nfT = np.ascontiguousarray(nf[b].T)  # [256, 512]
        if i0:
            nfT = np.ascontiguousarray(np.roll(nfT, -i0, axis=1))
        m = dict(common)
        m["nfT0"] = np.ascontiguousarray(nfT[0:128])
        m["nfT1"] = np.ascontiguousarray(nfT[128:256])
        in_maps.append(m)
    return in_maps


def _assemble(results):
    adjacency = np.empty((BS, N, N), np.float32)
    p0 = np.empty((BS, N, N), np.float32)
    emb = np.empty((BS, N, EMB), np.float32)
    for c in range(NCORES):
        b, i0 = c // 2, HALF * (c % 2)
        ac = np.asarray(results[c]["a_out"])
        pc = np.asarray(results[c]["p0_out"])
        if i0:
            ac = np.roll(ac, i0, axis=1)
            pc = np.roll(pc, i0, axis=1)
        adjacency[b, i0 : i0 + HALF] = ac
        p0[b, i0 : i0 + HALF] = pc
        emb[b, i0 : i0 + HALF] = np.asarray(results[c]["emb_out"])
    pred = np.stack([p0[:, _rows, _cols], adjacency[:, _rows, _cols]], axis=-1)
    prediction = np.ascontiguousarray(pred.reshape(BS, -1))
    return adjacency, prediction, emb


def kernel(**inputs):
    from concourse import bass_utils

    nc = _get_program()
    in_maps = _make_in_maps(inputs)
    res = bass_utils.run_bass_kernel_spmd(nc, in_maps, core_ids=list(range(NCORES)))
    return _assemble(res.results)


# revision 13
# speedup vs baseline: 1.5990x; 1.3750x over previous
# Trainium2 Bass kernel for FC_STGNN pedestrian edge-MLP (gnn_message_passing).
#
# Reference computation (BS=4, N=512, EMB=64):
#   h    = leaky(nf @ fc1_w + fc1_b)            [4,512,128]
#   emb  = leaky(h @ fc2_w + fc2_b)             [4,512,64]
#   edge_in[b,i,j] = [emb_i - emb_j, emb_i*emb_j]
#   eh   = relu(edge_in @ e1_w + e1_b)          [4,512,512,64]
#   logits = eh @ e2_w + e2_b                   [4,512,512,2]
#   edge_prob = softmax(logits)                 -> adjacency = p1, prediction = off-diag (p0,p1)
#
# Kernel restructuring:
#   * softmax over 2 classes == sigmoid of the logit difference:
#       d = eh @ (e2_w[:,1]-e2_w[:,0]) + (e2_b[1]-e2_b[0]);  p1 = sigmoid(d), p0 = sigmoid(-d)
#   * pre-activation factorization (W1 = e1_w[:64], W2 = e1_w[64:]):
#       pre[i,j,:] = emb_j @ (diag(emb_i) @ W2 - W1) + (emb_i @ W1 + e1_b)
#     For each pair of output rows (i, i+1) we build a [64,128] weight (two fused
#     vector ops), run one [64x128x512] fp32r matmul against the shared embT, and
#     absorb the per-i constant as the relu's per-partition bias (C2 trick below).
#   * the logit reduction runs with eh as the (bf16, FWL) stationary operand so the
#     [2,512] per-pair logits land as columns of a shared PSUM bank; sigmoid is then
#     batched over 64 pairs and PE-transposed back to row-major output chunks.
#
# Sharding: 8 cores; core c -> batch b=c//2, i-rows [i0, i0+256) with i0=256*(c%2).
# The per-core program is identical: the j axis is ROTATED by -i0 on the host
# (nfT input is np.roll'ed), so device columns j' correspond to real j=(j'+i0)%512
# and the core's own i-rows are always device columns 0..255. Host un-rotates.

import numpy as np

BS, N, EMB = 4, 512, 64
IN_DIM, HID = 256, 128
NCORES = 8
HALF = N // 2  # rows per core

_rows, _cols = np.nonzero(~np.eye(N, dtype=bool))

_COMPILED = {}


def _build_program():
    import concourse.tile as tile
    from concourse import bacc, mybir

    dt = mybir.dt
    f32 = dt.float32
    f32r = dt.float32r
    bf16 = dt.bfloat16
    AF = mybir.ActivationFunctionType
    OP = mybir.AluOpType

    nc = bacc.Bacc("TRN2", target_bir_lowering=False, debug=False)

    # ---- DRAM I/O (per-core shapes) ----
    nfT0 = nc.dram_tensor("nfT0", [128, N], f32, kind="ExternalInput").ap()
    nfT1 = nc.dram_tensor("nfT1", [128, N], f32, kind="ExternalInput").ap()
    fc1w0 = nc.dram_tensor("fc1w0", [128, HID], f32, kind="ExternalInput").ap()
    fc1w1 = nc.dram_tensor("fc1w1", [128, HID], f32, kind="ExternalInput").ap()
    fc2w = nc.dram_tensor("fc2w", [HID, EMB], f32, kind="ExternalInput").ap()
    w1 = nc.dram_tensor("w1", [EMB, EMB], f32, kind="ExternalInput").ap()
    w2 = nc.dram_tensor("w2", [EMB, EMB], f32, kind="ExternalInput").ap()
    # brow[0, 0:128]=fc1_b, [128:192]=fc2_b, [192:256]=e1_b
    brow = nc.dram_tensor("brow", [1, 256], f32, kind="ExternalInput").ap()
    # aux cols: 0/1 unused, 2=b_diff, 3=-b_diff (replicated down partitions)
    aux = nc.dram_tensor("aux", [128, 4], f32, kind="ExternalInput").ap()
    # aux2: [wd;0] and [0;wd] in bf16 (stationary side of the logit reduction)
    aux2 = nc.dram_tensor("aux2", [128, 2], bf16, kind="ExternalInput").ap()
    eye = nc.dram_tensor("eye", [128, 128], f32, kind="ExternalInput").ap()

    a_out = nc.dram_tensor("a_out", [HALF, N], f32, kind="ExternalOutput").ap()
    p0_out = nc.dram_tensor("p0_out", [HALF, N], f32, kind="ExternalOutput").ap()
    emb_out = nc.dram_tensor("emb_out", [HALF, EMB], f32, kind="ExternalOutput").ap()

    with tile.TileContext(nc) as tc:
        with (
            tc.tile_pool(name="const", bufs=1) as cpool,
            tc.tile_pool(name="wpair", bufs=6) as wpool,
            tc.tile_pool(name="eh", bufs=6) as ehpool,
            tc.tile_pool(name="sig", bufs=2) as sigpool,
            tc.tile_pool(name="chunk", bufs=2) as chpool,
            tc.tile_pool(name="psA", bufs=3, space="PSUM") as psA,
            tc.tile_pool(name="psL", bufs=2, space="PSUM") as psL,
            tc.tile_pool(name="psT", bufs=2, space="PSUM") as psT,
            tc.tile_pool(name="psP", bufs=1, space="PSUM") as psP,
        ):
            # ---- load constants ----
            t_nfT0 = cpool.tile([128, N], f32, tag="nfT0")
            t_nfT1 = cpool.tile([128, N], f32, tag="nfT1")
            t_fc1w0 = cpool.tile([128, HID], f32, tag="fc1w0")
            t_fc1w1 = cpool.tile([128, HID], f32, tag="fc1w1")
            t_fc2w = cpool.tile([HID, EMB], f32, tag="fc2w")
            t_w1 = cpool.tile([EMB, EMB], f32, tag="w1")
            t_w2 = cpool.tile([EMB, EMB], f32, tag="w2")
            t_brow = cpool.tile([1, 256], f32, tag="brow")
            t_aux = cpool.tile([128, 4], f32, tag="aux")
            t_aux2 = cpool.tile([128, 2], bf16, tag="aux2")
            t_eye = cpool.tile([128, 128], f32, tag="eye")
            nc.sync.dma_start(t_nfT0[:], nfT0)
            nc.sync.dma_start(t_nfT1[:], nfT1)
            nc.sync.dma_start(t_fc1w0[:], fc1w0)
            nc.sync.dma_start(t_fc1w1[:], fc1w1)
            nc.sync.dma_start(t_fc2w[:], fc2w)
            nc.sync.dma_start(t_w1[:], w1)
            nc.sync.dma_start(t_w2[:], w2)
            nc.sync.dma_start(t_brow[:], brow)
            nc.sync.dma_start(t_aux[:], aux)
            nc.sync.dma_start(t_aux2[:], aux2)
            nc.sync.dma_start(t_eye[:], eye)

            # DVE conversion copies serve two purposes: every DMA-fed matmul
            # operand gets a single-engine producer (matmuls support only ONE
            # semaphore wait), and the fp32 prep matmuls become f32r (1 cyc/row
            # instead of 4).
            t_nfT0r = cpool.tile([128, N], f32r, tag="nfT0r")
            t_nfT1r = cpool.tile([128, N], f32r, tag="nfT1r")
            t_fc1w0r = cpool.tile([128, HID], f32r, tag="fc1w0r")
            t_fc1w1r = cpool.tile([128, HID], f32r, tag="fc1w1r")
            t_fc2wr = cpool.tile([HID, EMB], f32r, tag="fc2wr")
            t_w1r = cpool.tile([EMB, EMB], f32r, tag="w1r")
            t_browr = cpool.tile([1, 256], f32r, tag="browr")
            nc.vector.tensor_copy(t_fc1w0r[:], t_fc1w0[:])
            nc.vector.tensor_copy(t_nfT0r[:], t_nfT0[:])
            nc.vector.tensor_copy(t_fc1w1r[:], t_fc1w1[:])
            nc.vector.tensor_copy(t_nfT1r[:], t_nfT1[:])
            nc.vector.tensor_copy(t_fc2wr[:], t_fc2w[:])
            nc.vector.tensor_copy(t_w1r[:], t_w1[:])
            nc.vector.tensor_copy(t_browr[:], t_brow[:])
            for t in (t_eye, t_aux2):
                nc.vector.tensor_copy(t[:], t[:])

            t_onesf = cpool.tile([1, N], f32, tag="onesf")
            nc.vector.memset(t_onesf[:], 1.0)
            t_ones = cpool.tile([1, N], f32r, tag="ones")
            nc.vector.tensor_copy(t_ones[:], t_onesf[:])

            def leaky(dst, src, scratch_pool):
                # max(x, 0.01x); only one PSUM operand allowed per instruction,
                # so stage 0.01x through SBUF first.
                t_s = scratch_pool.tile(
                    [src.partition_size(), src.free_size()], f32, tag="leak"
                )
                nc.vector.tensor_scalar_mul(t_s[:], src, 0.01)
                nc.vector.scalar_tensor_tensor(
                    dst, src, 0.0, t_s[:], op0=OP.add, op1=OP.max
                )

            # ---- fc head: hT = leaky(fc1_w.T @ nfT + fc1_b) ----
            ps_h = psP.tile([128, N], f32, tag="prep")
            nc.tensor.matmul(ps_h[:], t_fc1w0r[:], t_nfT0r[:], start=True, stop=False)
            nc.tensor.matmul(ps_h[:], t_fc1w1r[:], t_nfT1r[:], start=False, stop=False)
            nc.tensor.matmul(
                ps_h[:], t_browr[0:1, 0:128], t_ones[:], start=False, stop=True
            )
            t_hT = cpool.tile([128, N], f32r, tag="hT")
            leaky(t_hT[:], ps_h[:], ehpool)

            # ---- embT = leaky(fc2_w.T @ hT + fc2_b), with zero columns at 512+ ----
            ps_e = psP.tile([EMB, N], f32, tag="prep")
            nc.tensor.matmul(ps_e[:], t_fc2wr[:], t_hT[:], start=True, stop=False)
            nc.tensor.matmul(
                ps_e[:], t_browr[0:1, 128:192], t_ones[:], start=False, stop=True
            )
            t_embT = cpool.tile([EMB, N], f32r, tag="embT")
            leaky(t_embT[:], ps_e[:], ehpool)

            # ---- C2: relu-bias source. C2[0:64,c]=C(col c), C2[64:128,c]=C(col c+1)
            # where C(col) = W1.T @ embT[:,col] + e1_b ----
            ps_c = psP.tile([EMB, N], f32, tag="prep")
            nc.tensor.matmul(ps_c[:], t_w1r[:], t_embT[:], start=True, stop=False)
            nc.tensor.matmul(
                ps_c[:], t_browr[0:1, 192:256], t_ones[:], start=False, stop=True
            )
            t_Cs = cpool.tile([EMB, N], f32, tag="Cs")
            nc.scalar.copy(t_Cs[:], ps_c[:])
            t_C2 = cpool.tile([128, N], f32, tag="C2")
            nc.sync.dma_start(t_C2[0:EMB, :], t_Cs[:])
            nc.sync.dma_start(t_C2[EMB:128, 0 : N - 1], t_Cs[:, 1:N])

            # ---- emb output rows (this core's 256 rows = device cols 0..255) ----
            for k in range(2):
                ps_m = psP.tile([128, EMB], f32, tag="prep")
                nc.tensor.matmul(
                    ps_m[:],
                    t_hT[:, 128 * k : 128 * (k + 1)],
                    t_fc2wr[:],
                    start=True,
                    stop=False,
                )
                nc.tensor.matmul(
                    ps_m[:],
                    t_ones[0:1, 0:128],
                    t_browr[0:1, 128:192],
                    start=False,
                    stop=True,
                )
                t_m = ehpool.tile([128, EMB], f32, tag="embrow")
                leaky(t_m[:], ps_m[:], ehpool)
                nc.sync.dma_start(emb_out[128 * k : 128 * (k + 1), :], t_m[:])

            # ---- main loop: 128 pairs of i-rows, in 2 groups of 64 ----
            NPAIR = HALF // 2
            GROUP = 64
            for g in range(NPAIR // GROUP):
                # logitsT bank: column 128*jb + 2*pp + s = logit(i_loc=2*pp+s) for
                # j rows 128*jb..128*jb+127
                ps_log = psL.tile([128, N], f32, tag="logT")
                for pp in range(GROUP):
                    p = g * GROUP + pp
                    iA = 2 * p  # device column of first row of the pair

                    t_w = wpool.tile([EMB, 128], f32r, tag="wpair")
                    if True:
                        # Pool engine path: mult-with-broadcast + subtract
                        t_tmp = wpool.tile([EMB, 128], f32, tag="wtmp")
                        for q in range(2):
                            bc = t_embT[:, iA + q : iA + q + 1].broadcast_to([EMB, EMB])
                            nc.gpsimd.tensor_tensor(
                                t_tmp[:, EMB * q : EMB * (q + 1)],
                                t_w2[:],
                                bc,
                                op=OP.mult,
                            )
                            nc.gpsimd.tensor_tensor(
                                t_w[:, EMB * q : EMB * (q + 1)],
                                t_tmp[:, EMB * q : EMB * (q + 1)],
                                t_w1[:],
                                op=OP.subtract,
                            )
                    else:
                        for q in range(2):
                            nc.vector.scalar_tensor_tensor(
                                t_w[:, EMB * q : EMB * (q + 1)],
                                t_w2[:],
                                t_embT[:, iA + q : iA + q + 1],
                                t_w1[:],
                                op0=OP.mult,
                                op1=OP.subtract,
                            )

                    ps_pre = psA.tile([128, N], f32, tag="pre")
                    nc.tensor.matmul(
                        ps_pre[:],
                        t_w[:],
                        t_embT[:, 0:N],
                    )

                    t_eh = ehpool.tile([128, N], bf16, tag="eh")
                    if p % 2 == 0:
                        nc.scalar.activation(
                            t_eh[:],
                            ps_pre[:],
                            AF.Relu,
                            bias=t_C2[:, iA : iA + 1],
                            scale=1.0,
                        )
                    else:
                        nc.vector.tensor_scalar(
                            t_eh[:],
                            ps_pre[:],
                            t_C2[:, iA : iA + 1],
                            0.0,
                            op0=OP.add,
                            op1=OP.max,
                        )

                    for jb in range(4):
                        nc.tensor.matmul(
                            ps_log[:, 128 * jb + 2 * pp : 128 * jb + 2 * pp + 2],
                            t_eh[:, 128 * jb : 128 * (jb + 1)],
                            t_aux2[:],
                        )

                # batched sigmoid over the whole group (still j-major)
                t_p1T = sigpool.tile([128, N], f32, tag="p1T")
                t_p0T = sigpool.tile([128, N], f32, tag="p0T")
                nc.scalar.activation(
                    t_p1T[:], ps_log[:], AF.Sigmoid, bias=t_aux[:, 2:3], scale=1.0
                )
                nc.scalar.activation(
                    t_p0T[:], ps_log[:], AF.Sigmoid, bias=t_aux[:, 3:4], scale=-1.0
                )

                # transpose back to row-major [i_loc, j] chunks and ship out
                for s, (srcT, dram) in enumerate(((t_p0T, p0_out), (t_p1T, a_out))):
                    t_chunk = chpool.tile([128, N], f32, tag=f"chunk{s}")
                    for jb in range(4):
                        ps_t = psT.tile([128, 128], f32, tag="tr")
                        nc.tensor.transpose(
                            ps_t[:], srcT[:, 128 * jb : 128 * (jb + 1)], t_eye[:]
                        )
                        if s == 0:
                            nc.vector.tensor_copy(
                                t_chunk[:, 128 * jb : 128 * (jb + 1)], ps_t[:]
                            )
                        else:
                            nc.scalar.copy(
                                t_chunk[:, 128 * jb : 128 * (jb + 1)], ps_t[:]
                            )
                    nc.sync.dma_start(
                        dram[128 * g : 128 * (g + 1), :], t_chunk[:]
                    )

    nc.compile()
    return nc


def _get_program():
    if "nc" not in _COMPILED:
        _COMPILED["nc"] = _build_program()
    return _COMPILED["nc"]


def _make_in_maps(inputs):
    import ml_dtypes

    nf = np.asarray(inputs["node_features"], np.float32)
    fc1_w = np.asarray(inputs["fc1_w"], np.float32)
    fc1_b = np.asarray(inputs["fc1_b"], np.float32)
    fc2_w = np.asarray(inputs["fc2_w"], np.float32)
    fc2_b = np.asarray(inputs["fc2_b"], np.float32)
    e1_w = np.asarray(inputs["e1_w"], np.float32)
    e1_b = np.asarray(inputs["e1_b"], np.float32)
    e2_w = np.asarray(inputs["e2_w"], np.float32)
    e2_b = np.asarray(inputs["e2_b"], np.float32)

    wd = e2_w[:, 1] - e2_w[:, 0]  # [64]
    b_diff = float(e2_b[1] - e2_b[0])

    brow = np.zeros((1, 256), np.float32)
    brow[0, 0:128] = fc1_b
    brow[0, 128:192] = fc2_b
    brow[0, 192:256] = e1_b

    aux = np.zeros((128, 4), np.float32)
    aux[:, 2] = b_diff
    aux[:, 3] = -b_diff

    aux2 = np.zeros((128, 2), np.float32)
    aux2[0:64, 0] = wd
    aux2[64:128, 1] = wd
    aux2 = aux2.astype(ml_dtypes.bfloat16)

    common = {
        "fc1w0": np.ascontiguousarray(fc1_w[0:128]),
        "fc1w1": np.ascontiguousarray(fc1_w[128:256]),
        "fc2w": np.ascontiguousarray(fc2_w),
        "w1": np.ascontiguousarray(e1_w[0:64]),
        "w2": np.ascontiguousarray(e1_w[64:128]),
        "brow": brow,
        "aux": aux,
        "aux2": aux2,
        "eye": np.eye(128, dtype=np.float32),
    }

    in_maps = []
    for c in range(NCORES):
        b, i0 = c // 2, HALF * (c % 2)
        nfT = np.ascontiguousarray(nf[b].T)  # [256, 512]
        if i0:
            nfT = np.ascontiguousarray(np.roll(nfT, -i0, axis=1))
        m = dict(common)
        m["nfT0"] = np.ascontiguousarray(nfT[0:128])
        m["nfT1"] = np.ascontiguousarray(nfT[128:256])
        in_maps.append(m)
    return in_maps


def _assemble(results):
    adjacency = np.empty((BS, N, N), np.float32)
    p0 = np.empty((BS, N, N), np.float32)
    emb = np.empty((BS, N, EMB), np.float32)
    for c in range(NCORES):
        b, i0 = c // 2, HALF * (c % 2)
        ac = np.asarray(results[c]["a_out"])
        pc = np.asarray(results[c]["p0_out"])
        if i0:
            ac = np.roll(ac, i0, axis=1)
            pc = np.roll(pc, i0, axis=1)
        adjacency[b, i0 : i0 + HALF] = ac
        p0[b, i0 : i0 + HALF] = pc
        emb[b, i0 : i0 + HALF] = np.asarray(results[c]["emb_out"])
    pred = np.stack([p0[:, _rows, _cols], adjacency[:, _rows, _cols]], axis=-1)
    prediction = np.ascontiguousarray(pred.reshape(BS, -1))
    return adjacency, prediction, emb


def kernel(**inputs):
    from concourse import bass_utils

    nc = _get_program()
    in_maps = _make_in_maps(inputs)
    res = bass_utils.run_bass_kernel_spmd(nc, in_maps, core_ids=list(range(NCORES)))
    return _assemble(res.results)
